# revision 4
# baseline (speedup 1.0000x reference)
"""MoE (top-2 of 8 experts, SwiGLU MLP) Trainium2 kernel.

Strategy: expert parallelism across 8 NeuronCores. The (tiny) router runs
on host; tokens are gathered per expert on host and each core runs one
expert's SwiGLU MLP over its tokens with weights resident in SBUF.
Host applies the renormalized top-2 combine weights and scatter-adds the
two expert outputs per token.

Device layout: activations are kept transposed ([feature, token]) so every
matmul has its contraction dim on partitions with the weight tile
stationary; no on-device transposes are needed anywhere.
"""

import sys

import numpy as np

for _p in ("/root/.axon_site", "/root/.axon_site/_ro/trn_rl_repo",
           "/root/.axon_site/_ro/pypackages", "/opt/trn_rl_repo", "/opt/pypackages"):
    if _p not in sys.path:
        sys.path.append(_p)

import ml_dtypes  # noqa: E402

import concourse.bass as bass  # noqa: E402
import concourse.tile as tile  # noqa: E402
from concourse import bacc, mybir  # noqa: E402
from concourse.bass import ts  # noqa: E402
from concourse.bass_utils import run_bass_kernel_spmd  # noqa: E402

B, S, D, F, E, K = 4, 4096, 1024, 2048, 8, 2
N_CORES = 8
TOK_TILE = 512
BF16 = mybir.dt.bfloat16
F32 = mybir.dt.float32


def _build_nc(tcap: int) -> bass.Bass:
    """One expert's SwiGLU MLP over tcap (padded) tokens, transposed layout."""
    nd = D // 128   # 8 d-chunks
    nf = F // 128   # 16 f-chunks
    nt = tcap // TOK_TILE

    nc = bacc.Bacc("TRN2", debug=False, target_bir_lowering=False,
                   num_devices=N_CORES)
    xt = nc.dram_tensor("xt", [D, tcap], BF16, kind="ExternalInput").ap()
    wg = nc.dram_tensor("wg", [D, F], BF16, kind="ExternalInput").ap()
    wu = nc.dram_tensor("wu", [D, F], BF16, kind="ExternalInput").ap()
    wd = nc.dram_tensor("wd", [F, D], BF16, kind="ExternalInput").ap()
    yt = nc.dram_tensor("yt", [D, tcap], F32, kind="ExternalOutput").ap()

    with tile.TileContext(nc) as tc:
        with tc.tile_pool(name="wpool", bufs=1) as wpool, \
             tc.tile_pool(name="xpool", bufs=3) as xpool, \
             tc.tile_pool(name="hpool", bufs=2) as hpool, \
             tc.tile_pool(name="spool", bufs=3) as spool, \
             tc.tile_pool(name="opool", bufs=3) as opool, \
             tc.tile_pool(name="gp", bufs=2, space="PSUM") as gp, \
             tc.tile_pool(name="up", bufs=2, space="PSUM") as up, \
             tc.tile_pool(name="yp", bufs=2, space="PSUM") as yp:

            wg_sb = wpool.tile([128, nd * F], BF16)
            wu_sb = wpool.tile([128, nd * F], BF16)
            wd_sb = wpool.tile([128, nf * D], BF16)
            for c in range(nd):
                nc.sync.dma_start(wg_sb[:, ts(c, F)], wg[ts(c, 128), :])
                nc.sync.dma_start(wu_sb[:, ts(c, F)], wu[ts(c, 128), :])
            for c in range(nf):
                nc.sync.dma_start(wd_sb[:, ts(c, D)], wd[ts(c, 128), :])

            for j in range(nt):
                x_sb = xpool.tile([128, nd * TOK_TILE], BF16)
                for c in range(nd):
                    nc.sync.dma_start(x_sb[:, ts(c, TOK_TILE)],
                                      xt[ts(c, 128), ts(j, TOK_TILE)])
                h_sb = hpool.tile([128, nf * TOK_TILE], BF16)
                for f in range(nf):
                    g_ps = gp.tile([128, TOK_TILE], F32)
                    for c in range(nd):
                        nc.tensor.matmul(
                            g_ps[:],
                            wg_sb[:, c * F + f * 128: c * F + (f + 1) * 128],
                            x_sb[:, ts(c, TOK_TILE)],
                            start=(c == 0), stop=(c == nd - 1))
                    u_ps = up.tile([128, TOK_TILE], F32)
                    for c in range(nd):
                        nc.tensor.matmul(
                            u_ps[:],
                            wu_sb[:, c * F + f * 128: c * F + (f + 1) * 128],
                            x_sb[:, ts(c, TOK_TILE)],
                            start=(c == 0), stop=(c == nd - 1))
                    s_sb = spool.tile([128, TOK_TILE], F32)
                    nc.scalar.activation(s_sb[:], g_ps[:],
                                         mybir.ActivationFunctionType.Silu)
                    nc.vector.tensor_mul(h_sb[:, ts(f, TOK_TILE)], s_sb[:], u_ps[:])
                for dm in range(nd):
                    y_ps = yp.tile([128, TOK_TILE], F32)
                    for f in range(nf):
                        nc.tensor.matmul(
                            y_ps[:],
                            wd_sb[:, f * D + dm * 128: f * D + (dm + 1) * 128],
                            h_sb[:, ts(f, TOK_TILE)],
                            start=(f == 0), stop=(f == nf - 1))
                    y_sb = opool.tile([128, TOK_TILE], F32)
                    nc.vector.tensor_copy(y_sb[:], y_ps[:])
                    nc.sync.dma_start(yt[ts(dm, 128), ts(j, TOK_TILE)], y_sb[:])
    nc.compile()
    return nc


def _route(x: np.ndarray, router_w: np.ndarray):
    """Host router identical in math to the jax reference (fp32)."""
    logits = x @ router_w.T                                   # [T, E]
    logits = logits - logits.max(axis=-1, keepdims=True)
    ex = np.exp(logits, dtype=np.float32)
    scores = ex / ex.sum(axis=-1, keepdims=True)              # [T, E]
    topk_idx = np.argsort(-scores, axis=-1, kind="stable")[:, :K]   # [T, K]
    topk_w = np.take_along_axis(scores, topk_idx, axis=-1)
    topk_w = topk_w / topk_w.sum(axis=-1, keepdims=True)
    return topk_idx.astype(np.int64), topk_w.astype(np.float32)


_NC_CACHE: dict[int, bass.Bass] = {}


def _run_device(in_maps, tcap, trace=False, **kw):
    nc = _NC_CACHE.get(tcap)
    if nc is None:
        nc = _build_nc(tcap)
        _NC_CACHE[tcap] = nc
    return run_bass_kernel_spmd(nc, in_maps, core_ids=list(range(N_CORES)),
                                trace=trace, **kw)


def _prepare(hidden_states, router_w, w_gate, w_up, w_down):
    x = np.ascontiguousarray(hidden_states.reshape(-1, D)).astype(np.float32)
    topk_idx, topk_w = _route(x, router_w.astype(np.float32))

    tok_lists, w_lists = [], []
    for e in range(E):
        mask = topk_idx == e                                   # [T, K]
        tok_e = np.nonzero(mask.any(axis=1))[0]
        w_e = (topk_w * mask)[tok_e].sum(axis=1).astype(np.float32)
        tok_lists.append(tok_e)
        w_lists.append(w_e)

    max_count = max(len(t) for t in tok_lists)
    tcap = -(-max_count // TOK_TILE) * TOK_TILE

    in_maps = []
    for e in range(E):
        xt = np.zeros((D, tcap), dtype=ml_dtypes.bfloat16)
        xe = x[tok_lists[e]]                                   # [count, D] f32
        xt[:, :len(tok_lists[e])] = xe.T.astype(ml_dtypes.bfloat16)
        in_maps.append({
            "xt": xt,
            "wg": np.ascontiguousarray(w_gate[e].T).astype(ml_dtypes.bfloat16),
            "wu": np.ascontiguousarray(w_up[e].T).astype(ml_dtypes.bfloat16),
            "wd": np.ascontiguousarray(w_down[e].T).astype(ml_dtypes.bfloat16),
        })
    return in_maps, tok_lists, w_lists, tcap


def _combine(results, tok_lists, w_lists):
    out = np.zeros((B * S, D), dtype=np.float32)
    for e in range(E):
        yt = results[e]["yt"]                                  # [D, tcap] f32
        n = len(tok_lists[e])
        out[tok_lists[e]] += w_lists[e][:, None] * yt[:, :n].T
    return out.reshape(B, S, D)


def kernel(hidden_states, router_w, w_gate, w_up, w_down):
    in_maps, tok_lists, w_lists, tcap = _prepare(
        hidden_states, router_w, w_gate, w_up, w_down)
    res = _run_device(in_maps, tcap)
    return _combine(res.results, tok_lists, w_lists)


def kernel_traced(hidden_states, router_w, w_gate, w_up, w_down, **kw):
    """Same as kernel() but returns (output, BassKernelResults) with NTFF trace."""
    in_maps, tok_lists, w_lists, tcap = _prepare(
        hidden_states, router_w, w_gate, w_up, w_down)
    res = _run_device(in_maps, tcap, trace=True, **kw)
    return _combine(res.results, tok_lists, w_lists), res


# revision 7
# speedup vs baseline: 1.1123x; 1.1123x over previous
"""MoE (top-2 of 8 experts, SwiGLU MLP) Trainium2 kernel.

Strategy: expert parallelism across 8 NeuronCores. The (tiny) router runs
on host; tokens are gathered per expert on host and each core runs one
expert's SwiGLU MLP over its tokens with weights resident in SBUF.
Host applies the renormalized top-2 combine weights and scatter-adds the
two expert outputs per token.

Device layout: activations are kept transposed ([feature, token]) so every
matmul has its contraction dim on partitions with the weight tile
stationary; no on-device transposes are needed anywhere.
"""

import sys

import numpy as np

for _p in ("/root/.axon_site", "/root/.axon_site/_ro/trn_rl_repo",
           "/root/.axon_site/_ro/pypackages", "/opt/trn_rl_repo", "/opt/pypackages"):
    if _p not in sys.path:
        sys.path.append(_p)

import ml_dtypes  # noqa: E402

import concourse.bass as bass  # noqa: E402
import concourse.tile as tile  # noqa: E402
from concourse import bacc, mybir  # noqa: E402
from concourse.bass import ts  # noqa: E402
from concourse.bass_utils import run_bass_kernel_spmd  # noqa: E402

B, S, D, F, E, K = 4, 4096, 1024, 2048, 8, 2
N_CORES = 8
TOK_TILE = 512
TOK_ALIGN = 128
ACT_DT = mybir.dt.float16
ACT_NP = np.float16
F32 = mybir.dt.float32


def _build_nc(tcap: int) -> bass.Bass:
    """One expert's SwiGLU MLP over tcap (padded) tokens, transposed layout."""
    nd = D // 128   # 8 d-chunks
    nf = F // 128   # 16 f-chunks
    # full 512-token tiles plus one remainder tile (multiple of 128)
    tiles = [(o, min(TOK_TILE, tcap - o)) for o in range(0, tcap, TOK_TILE)]

    nc = bacc.Bacc("TRN2", debug=False, target_bir_lowering=False,
                   num_devices=N_CORES)
    xt = nc.dram_tensor("xt", [D, tcap], ACT_DT, kind="ExternalInput").ap()
    wg = nc.dram_tensor("wg", [D, F], ACT_DT, kind="ExternalInput").ap()
    wu = nc.dram_tensor("wu", [D, F], ACT_DT, kind="ExternalInput").ap()
    wd = nc.dram_tensor("wd", [F, D], ACT_DT, kind="ExternalInput").ap()
    yt = nc.dram_tensor("yt", [D, tcap], F32, kind="ExternalOutput").ap()

    with tile.TileContext(nc) as tc:
        with tc.tile_pool(name="wpool", bufs=1) as wpool, \
             tc.tile_pool(name="xpool", bufs=3) as xpool, \
             tc.tile_pool(name="hpool", bufs=2) as hpool, \
             tc.tile_pool(name="spool", bufs=3) as spool, \
             tc.tile_pool(name="opool", bufs=3) as opool, \
             tc.tile_pool(name="gp", bufs=2, space="PSUM") as gp, \
             tc.tile_pool(name="up", bufs=2, space="PSUM") as up, \
             tc.tile_pool(name="yp", bufs=2, space="PSUM") as yp:

            wg_sb = wpool.tile([128, nd * F], ACT_DT)
            wu_sb = wpool.tile([128, nd * F], ACT_DT)
            wd_sb = wpool.tile([128, nf * D], ACT_DT)
            # first x tile + gate weights first: they gate the first matmuls
            x_tiles = {}
            x_tiles[0] = xpool.tile([128, nd * TOK_TILE], ACT_DT, tag="x", name="x_sb0")
            for c in range(nd):
                nc.sync.dma_start(x_tiles[0][:, c * TOK_TILE: c * TOK_TILE + tiles[0][1]],
                                  xt[ts(c, 128), tiles[0][0]: tiles[0][0] + tiles[0][1]])
            for c in range(nd):
                nc.sync.dma_start(wg_sb[:, ts(c, F)], wg[ts(c, 128), :])
            for c in range(nd):
                nc.sync.dma_start(wu_sb[:, ts(c, F)], wu[ts(c, 128), :])
            for c in range(nf):
                nc.sync.dma_start(wd_sb[:, ts(c, D)], wd[ts(c, 128), :])

            for j, (off, w) in enumerate(tiles):
                x_sb = x_tiles.get(j)
                if x_sb is None:
                    x_sb = xpool.tile([128, nd * TOK_TILE], ACT_DT, tag="x", name=f"x_sb{j}")
                    for c in range(nd):
                        nc.sync.dma_start(x_sb[:, c * TOK_TILE: c * TOK_TILE + w],
                                          xt[ts(c, 128), off: off + w])
                h_sb = hpool.tile([128, nf * TOK_TILE], ACT_DT)
                for f in range(nf):
                    g_ps = gp.tile([128, TOK_TILE], F32)
                    for c in range(nd):
                        nc.tensor.matmul(
                            g_ps[:, :w],
                            wg_sb[:, c * F + f * 128: c * F + (f + 1) * 128],
                            x_sb[:, c * TOK_TILE: c * TOK_TILE + w],
                            start=(c == 0), stop=(c == nd - 1))
                    u_ps = up.tile([128, TOK_TILE], F32)
                    for c in range(nd):
                        nc.tensor.matmul(
                            u_ps[:, :w],
                            wu_sb[:, c * F + f * 128: c * F + (f + 1) * 128],
                            x_sb[:, c * TOK_TILE: c * TOK_TILE + w],
                            start=(c == 0), stop=(c == nd - 1))
                    s_sb = spool.tile([128, TOK_TILE], F32)
                    nc.scalar.activation(s_sb[:, :w], g_ps[:, :w],
                                         mybir.ActivationFunctionType.Silu)
                    nc.vector.tensor_mul(h_sb[:, f * TOK_TILE: f * TOK_TILE + w],
                                         s_sb[:, :w], u_ps[:, :w])
                for dm in range(nd):
                    y_ps = yp.tile([128, TOK_TILE], F32)
                    for f in range(nf):
                        nc.tensor.matmul(
                            y_ps[:, :w],
                            wd_sb[:, f * D + dm * 128: f * D + (dm + 1) * 128],
                            h_sb[:, f * TOK_TILE: f * TOK_TILE + w],
                            start=(f == 0), stop=(f == nf - 1))
                    y_sb = opool.tile([128, TOK_TILE], F32)
                    nc.vector.tensor_copy(y_sb[:, :w], y_ps[:, :w])
                    nc.sync.dma_start(yt[ts(dm, 128), off: off + w], y_sb[:, :w])
    nc.compile()
    return nc


def _route(x: np.ndarray, router_w: np.ndarray):
    """Host router identical in math to the jax reference (fp32)."""
    logits = x @ router_w.T                                   # [T, E]
    logits = logits - logits.max(axis=-1, keepdims=True)
    ex = np.exp(logits, dtype=np.float32)
    scores = ex / ex.sum(axis=-1, keepdims=True)              # [T, E]
    topk_idx = np.argsort(-scores, axis=-1, kind="stable")[:, :K]   # [T, K]
    topk_w = np.take_along_axis(scores, topk_idx, axis=-1)
    topk_w = topk_w / topk_w.sum(axis=-1, keepdims=True)
    return topk_idx.astype(np.int64), topk_w.astype(np.float32)


_NC_CACHE: dict[int, bass.Bass] = {}


def _run_device(in_maps, tcap, trace=False, **kw):
    nc = _NC_CACHE.get(tcap)
    if nc is None:
        nc = _build_nc(tcap)
        _NC_CACHE[tcap] = nc
    return run_bass_kernel_spmd(nc, in_maps, core_ids=list(range(N_CORES)),
                                trace=trace, **kw)


def _prepare(hidden_states, router_w, w_gate, w_up, w_down):
    x = np.ascontiguousarray(hidden_states.reshape(-1, D)).astype(np.float32)
    topk_idx, topk_w = _route(x, router_w.astype(np.float32))

    tok_lists, w_lists = [], []
    for e in range(E):
        mask = topk_idx == e                                   # [T, K]
        tok_e = np.nonzero(mask.any(axis=1))[0]
        w_e = (topk_w * mask)[tok_e].sum(axis=1).astype(np.float32)
        tok_lists.append(tok_e)
        w_lists.append(w_e)

    max_count = max(len(t) for t in tok_lists)
    tcap = -(-max_count // TOK_ALIGN) * TOK_ALIGN

    in_maps = []
    for e in range(E):
        xt = np.zeros((D, tcap), dtype=ACT_NP)
        xe = x[tok_lists[e]]                                   # [count, D] f32
        xt[:, :len(tok_lists[e])] = xe.T.astype(ACT_NP)
        in_maps.append({
            "xt": xt,
            "wg": np.ascontiguousarray(w_gate[e].T).astype(ACT_NP),
            "wu": np.ascontiguousarray(w_up[e].T).astype(ACT_NP),
            "wd": np.ascontiguousarray(w_down[e].T).astype(ACT_NP),
        })
    return in_maps, tok_lists, w_lists, tcap


def _combine(results, tok_lists, w_lists):
    out = np.zeros((B * S, D), dtype=np.float32)
    for e in range(E):
        yt = results[e]["yt"]                                  # [D, tcap] f32
        n = len(tok_lists[e])
        out[tok_lists[e]] += w_lists[e][:, None] * yt[:, :n].T
    return out.reshape(B, S, D)


def kernel(hidden_states, router_w, w_gate, w_up, w_down):
    in_maps, tok_lists, w_lists, tcap = _prepare(
        hidden_states, router_w, w_gate, w_up, w_down)
    res = _run_device(in_maps, tcap)
    return _combine(res.results, tok_lists, w_lists)


def kernel_traced(hidden_states, router_w, w_gate, w_up, w_down, **kw):
    """Same as kernel() but returns (output, BassKernelResults) with NTFF trace."""
    in_maps, tok_lists, w_lists, tcap = _prepare(
        hidden_states, router_w, w_gate, w_up, w_down)
    res = _run_device(in_maps, tcap, trace=True, **kw)
    return _combine(res.results, tok_lists, w_lists), res


# revision 10
# speedup vs baseline: 1.1255x; 1.0119x over previous
"""MoE (top-2 of 8 experts, SwiGLU MLP) Trainium2 kernel.

Strategy: expert parallelism across 8 NeuronCores. The (tiny) router runs
on host; tokens are gathered per expert on host and each core runs one
expert's SwiGLU MLP over its tokens with weights resident in SBUF.
Host applies the renormalized top-2 combine weights and scatter-adds the
two expert outputs per token.

Device layout: activations are kept transposed ([feature, token]) so every
matmul has its contraction dim on partitions with the weight tile
stationary; no on-device transposes are needed anywhere.
"""

import sys

import numpy as np

for _p in ("/root/.axon_site", "/root/.axon_site/_ro/trn_rl_repo",
           "/root/.axon_site/_ro/pypackages", "/opt/trn_rl_repo", "/opt/pypackages"):
    if _p not in sys.path:
        sys.path.append(_p)

import ml_dtypes  # noqa: E402

import concourse.bass as bass  # noqa: E402
import concourse.tile as tile  # noqa: E402
from concourse import bacc, mybir  # noqa: E402
from concourse.bass import ts  # noqa: E402
from concourse.bass_utils import run_bass_kernel_spmd  # noqa: E402

B, S, D, F, E, K = 4, 4096, 1024, 2048, 8, 2
N_CORES = 8
TOK_TILE = 512
TOK_ALIGN = 32
ACT_DT = mybir.dt.float16
ACT_NP = np.float16
F32 = mybir.dt.float32


def _build_nc(tcap: int) -> bass.Bass:
    """One expert's SwiGLU MLP over tcap (padded) tokens, transposed layout."""
    nd = D // 128   # 8 d-chunks
    nf = F // 128   # 16 f-chunks
    # full 512-token tiles plus one remainder tile (multiple of 128)
    tiles = [(o, min(TOK_TILE, tcap - o)) for o in range(0, tcap, TOK_TILE)]

    nc = bacc.Bacc("TRN2", debug=False, target_bir_lowering=False,
                   num_devices=N_CORES)
    xt = nc.dram_tensor("xt", [D, tcap], ACT_DT, kind="ExternalInput").ap()
    wg = nc.dram_tensor("wg", [D, F], ACT_DT, kind="ExternalInput").ap()
    wu = nc.dram_tensor("wu", [D, F], ACT_DT, kind="ExternalInput").ap()
    wd = nc.dram_tensor("wd", [F, D], ACT_DT, kind="ExternalInput").ap()
    yt = nc.dram_tensor("yt", [D, tcap], F32, kind="ExternalOutput").ap()

    with tile.TileContext(nc) as tc:
        with tc.tile_pool(name="wpool", bufs=1) as wpool, \
             tc.tile_pool(name="xpool", bufs=3) as xpool, \
             tc.tile_pool(name="hpool", bufs=2) as hpool, \
             tc.tile_pool(name="spool", bufs=3) as spool, \
             tc.tile_pool(name="opool", bufs=3) as opool, \
             tc.tile_pool(name="gp", bufs=2, space="PSUM") as gp, \
             tc.tile_pool(name="up", bufs=2, space="PSUM") as up, \
             tc.tile_pool(name="yp", bufs=2, space="PSUM") as yp:

            # one tile per 128-row chunk so the first accumulation chain can
            # start as soon as its own chunk's DMA lands, not all of them
            wg_sb = [wpool.tile([128, F], ACT_DT, name=f"wg_sb{c}") for c in range(nd)]
            wu_sb = [wpool.tile([128, F], ACT_DT, name=f"wu_sb{c}") for c in range(nd)]
            wd_sb = [wpool.tile([128, D], ACT_DT, name=f"wd_sb{c}") for c in range(nf)]
            # first x tile interleaved with gate weights: they gate the first MMs
            x_tiles = {}
            x_tiles[0] = xpool.tile([128, nd * TOK_TILE], ACT_DT, tag="x", name="x_sb0")
            for c in range(nd):
                nc.sync.dma_start(x_tiles[0][:, c * TOK_TILE: c * TOK_TILE + tiles[0][1]],
                                  xt[ts(c, 128), tiles[0][0]: tiles[0][0] + tiles[0][1]])
                nc.sync.dma_start(wg_sb[c][:], wg[ts(c, 128), :])
            for c in range(nd):
                nc.sync.dma_start(wu_sb[c][:], wu[ts(c, 128), :])
            for c in range(nf):
                nc.sync.dma_start(wd_sb[c][:], wd[ts(c, 128), :])

            for j, (off, w) in enumerate(tiles):
                x_sb = x_tiles.get(j)
                if x_sb is None:
                    x_sb = xpool.tile([128, nd * TOK_TILE], ACT_DT, tag="x", name=f"x_sb{j}")
                    for c in range(nd):
                        nc.sync.dma_start(x_sb[:, c * TOK_TILE: c * TOK_TILE + w],
                                          xt[ts(c, 128), off: off + w])
                h_sb = hpool.tile([128, nf * TOK_TILE], ACT_DT)
                for f in range(nf):
                    g_ps = gp.tile([128, TOK_TILE], F32)
                    for c in range(nd):
                        nc.tensor.matmul(
                            g_ps[:, :w],
                            wg_sb[c][:, ts(f, 128)],
                            x_sb[:, c * TOK_TILE: c * TOK_TILE + w],
                            start=(c == 0), stop=(c == nd - 1))
                    u_ps = up.tile([128, TOK_TILE], F32)
                    for c in range(nd):
                        nc.tensor.matmul(
                            u_ps[:, :w],
                            wu_sb[c][:, ts(f, 128)],
                            x_sb[:, c * TOK_TILE: c * TOK_TILE + w],
                            start=(c == 0), stop=(c == nd - 1))
                    s_sb = spool.tile([128, TOK_TILE], F32)
                    nc.scalar.activation(s_sb[:, :w], g_ps[:, :w],
                                         mybir.ActivationFunctionType.Silu)
                    nc.vector.tensor_mul(h_sb[:, f * TOK_TILE: f * TOK_TILE + w],
                                         s_sb[:, :w], u_ps[:, :w])
                for dm in range(nd):
                    y_ps = yp.tile([128, TOK_TILE], F32)
                    for f in range(nf):
                        nc.tensor.matmul(
                            y_ps[:, :w],
                            wd_sb[f][:, ts(dm, 128)],
                            h_sb[:, f * TOK_TILE: f * TOK_TILE + w],
                            start=(f == 0), stop=(f == nf - 1))
                    y_sb = opool.tile([128, TOK_TILE], F32)
                    nc.vector.tensor_copy(y_sb[:, :w], y_ps[:, :w])
                    nc.sync.dma_start(yt[ts(dm, 128), off: off + w], y_sb[:, :w])
    nc.compile()
    return nc


def _route(x: np.ndarray, router_w: np.ndarray):
    """Host router identical in math to the jax reference (fp32)."""
    logits = x @ router_w.T                                   # [T, E]
    logits = logits - logits.max(axis=-1, keepdims=True)
    ex = np.exp(logits, dtype=np.float32)
    scores = ex / ex.sum(axis=-1, keepdims=True)              # [T, E]
    topk_idx = np.argsort(-scores, axis=-1, kind="stable")[:, :K]   # [T, K]
    topk_w = np.take_along_axis(scores, topk_idx, axis=-1)
    topk_w = topk_w / topk_w.sum(axis=-1, keepdims=True)
    return topk_idx.astype(np.int64), topk_w.astype(np.float32)


_NC_CACHE: dict[int, bass.Bass] = {}


def _run_device(in_maps, tcap, trace=False, **kw):
    nc = _NC_CACHE.get(tcap)
    if nc is None:
        nc = _build_nc(tcap)
        _NC_CACHE[tcap] = nc
    return run_bass_kernel_spmd(nc, in_maps, core_ids=list(range(N_CORES)),
                                trace=trace, **kw)


def _prepare(hidden_states, router_w, w_gate, w_up, w_down):
    x = np.ascontiguousarray(hidden_states.reshape(-1, D)).astype(np.float32)
    topk_idx, topk_w = _route(x, router_w.astype(np.float32))

    tok_lists, w_lists = [], []
    for e in range(E):
        mask = topk_idx == e                                   # [T, K]
        tok_e = np.nonzero(mask.any(axis=1))[0]
        w_e = (topk_w * mask)[tok_e].sum(axis=1).astype(np.float32)
        tok_lists.append(tok_e)
        w_lists.append(w_e)

    max_count = max(len(t) for t in tok_lists)
    tcap = -(-max_count // TOK_ALIGN) * TOK_ALIGN

    in_maps = []
    for e in range(E):
        xt = np.zeros((D, tcap), dtype=ACT_NP)
        xe = x[tok_lists[e]]                                   # [count, D] f32
        xt[:, :len(tok_lists[e])] = xe.T.astype(ACT_NP)
        in_maps.append({
            "xt": xt,
            "wg": np.ascontiguousarray(w_gate[e].T).astype(ACT_NP),
            "wu": np.ascontiguousarray(w_up[e].T).astype(ACT_NP),
            "wd": np.ascontiguousarray(w_down[e].T).astype(ACT_NP),
        })
    return in_maps, tok_lists, w_lists, tcap


def _combine(results, tok_lists, w_lists):
    out = np.zeros((B * S, D), dtype=np.float32)
    for e in range(E):
        yt = results[e]["yt"]                                  # [D, tcap] f32
        n = len(tok_lists[e])
        out[tok_lists[e]] += w_lists[e][:, None] * yt[:, :n].T
    return out.reshape(B, S, D)


def kernel(hidden_states, router_w, w_gate, w_up, w_down):
    in_maps, tok_lists, w_lists, tcap = _prepare(
        hidden_states, router_w, w_gate, w_up, w_down)
    res = _run_device(in_maps, tcap)
    return _combine(res.results, tok_lists, w_lists)


def kernel_traced(hidden_states, router_w, w_gate, w_up, w_down, **kw):
    """Same as kernel() but returns (output, BassKernelResults) with NTFF trace."""
    in_maps, tok_lists, w_lists, tcap = _prepare(
        hidden_states, router_w, w_gate, w_up, w_down)
    res = _run_device(in_maps, tcap, trace=True, **kw)
    return _combine(res.results, tok_lists, w_lists), res


# revision 11
# speedup vs baseline: 1.1288x; 1.0029x over previous
"""MoE (top-2 of 8 experts, SwiGLU MLP) Trainium2 kernel.

Strategy: expert parallelism across 8 NeuronCores. The (tiny) router runs
on host; tokens are gathered per expert on host and each core runs one
expert's SwiGLU MLP over its tokens with weights resident in SBUF.
Host applies the renormalized top-2 combine weights and scatter-adds the
two expert outputs per token.

Device layout: activations are kept transposed ([feature, token]) so every
matmul has its contraction dim on partitions with the weight tile
stationary; no on-device transposes are needed anywhere.
"""

import sys

import numpy as np

for _p in ("/root/.axon_site", "/root/.axon_site/_ro/trn_rl_repo",
           "/root/.axon_site/_ro/pypackages", "/opt/trn_rl_repo", "/opt/pypackages"):
    if _p not in sys.path:
        sys.path.append(_p)

import ml_dtypes  # noqa: E402

import concourse.bass as bass  # noqa: E402
import concourse.tile as tile  # noqa: E402
from concourse import bacc, mybir  # noqa: E402
from concourse.bass import ts  # noqa: E402
from concourse.bass_utils import run_bass_kernel_spmd  # noqa: E402

B, S, D, F, E, K = 4, 4096, 1024, 2048, 8, 2
N_CORES = 8
TOK_TILE = 512
TOK_ALIGN = 32
ACT_DT = mybir.dt.float16
ACT_NP = np.float16
F32 = mybir.dt.float32


def _build_nc(tcap: int) -> bass.Bass:
    """One expert's SwiGLU MLP over tcap (padded) tokens, transposed layout."""
    nd = D // 128   # 8 d-chunks
    nf = F // 128   # 16 f-chunks
    # full 512-token tiles; a short remainder is equalized across the last
    # two tiles (N=64 matmuls are NX-issue-bound at ~60ns, N>=256 are not)
    nfull, rem = divmod(tcap, TOK_TILE)
    widths = [TOK_TILE] * nfull
    if rem:
        if nfull:
            last_two = TOK_TILE + rem
            a = (last_two // 2 + 31) // 32 * 32
            widths = [TOK_TILE] * (nfull - 1) + [a, last_two - a]
        else:
            widths = [rem]
    tiles = []
    o = 0
    for w in widths:
        tiles.append((o, w))
        o += w

    nc = bacc.Bacc("TRN2", debug=False, target_bir_lowering=False,
                   num_devices=N_CORES)
    xt = nc.dram_tensor("xt", [D, tcap], ACT_DT, kind="ExternalInput").ap()
    wg = nc.dram_tensor("wg", [D, F], ACT_DT, kind="ExternalInput").ap()
    wu = nc.dram_tensor("wu", [D, F], ACT_DT, kind="ExternalInput").ap()
    wd = nc.dram_tensor("wd", [F, D], ACT_DT, kind="ExternalInput").ap()
    yt = nc.dram_tensor("yt", [D, tcap], F32, kind="ExternalOutput").ap()

    with tile.TileContext(nc) as tc:
        with tc.tile_pool(name="wpool", bufs=1) as wpool, \
             tc.tile_pool(name="xpool", bufs=3) as xpool, \
             tc.tile_pool(name="hpool", bufs=2) as hpool, \
             tc.tile_pool(name="spool", bufs=3) as spool, \
             tc.tile_pool(name="opool", bufs=3) as opool, \
             tc.tile_pool(name="gp", bufs=2, space="PSUM") as gp, \
             tc.tile_pool(name="up", bufs=2, space="PSUM") as up, \
             tc.tile_pool(name="yp", bufs=2, space="PSUM") as yp:

            # one tile per 128-row chunk so the first accumulation chain can
            # start as soon as its own chunk's DMA lands, not all of them
            wg_sb = [wpool.tile([128, F], ACT_DT, name=f"wg_sb{c}") for c in range(nd)]
            wu_sb = [wpool.tile([128, F], ACT_DT, name=f"wu_sb{c}") for c in range(nd)]
            wd_sb = [wpool.tile([128, D], ACT_DT, name=f"wd_sb{c}") for c in range(nf)]
            # first x tile interleaved with gate weights: they gate the first MMs
            x_tiles = {}
            x_tiles[0] = xpool.tile([128, nd * TOK_TILE], ACT_DT, tag="x", name="x_sb0")
            for c in range(nd):
                nc.sync.dma_start(x_tiles[0][:, c * TOK_TILE: c * TOK_TILE + tiles[0][1]],
                                  xt[ts(c, 128), tiles[0][0]: tiles[0][0] + tiles[0][1]])
                nc.sync.dma_start(wg_sb[c][:], wg[ts(c, 128), :])
            for c in range(nd):
                nc.sync.dma_start(wu_sb[c][:], wu[ts(c, 128), :])
            for c in range(nf):
                nc.sync.dma_start(wd_sb[c][:], wd[ts(c, 128), :])

            for j, (off, w) in enumerate(tiles):
                x_sb = x_tiles.get(j)
                if x_sb is None:
                    x_sb = xpool.tile([128, nd * TOK_TILE], ACT_DT, tag="x", name=f"x_sb{j}")
                    for c in range(nd):
                        nc.sync.dma_start(x_sb[:, c * TOK_TILE: c * TOK_TILE + w],
                                          xt[ts(c, 128), off: off + w])
                h_sb = hpool.tile([128, nf * TOK_TILE], ACT_DT)
                for f in range(nf):
                    g_ps = gp.tile([128, TOK_TILE], F32)
                    for c in range(nd):
                        nc.tensor.matmul(
                            g_ps[:, :w],
                            wg_sb[c][:, ts(f, 128)],
                            x_sb[:, c * TOK_TILE: c * TOK_TILE + w],
                            start=(c == 0), stop=(c == nd - 1))
                    u_ps = up.tile([128, TOK_TILE], F32)
                    for c in range(nd):
                        nc.tensor.matmul(
                            u_ps[:, :w],
                            wu_sb[c][:, ts(f, 128)],
                            x_sb[:, c * TOK_TILE: c * TOK_TILE + w],
                            start=(c == 0), stop=(c == nd - 1))
                    s_sb = spool.tile([128, TOK_TILE], F32)
                    nc.scalar.activation(s_sb[:, :w], g_ps[:, :w],
                                         mybir.ActivationFunctionType.Silu)
                    nc.vector.tensor_mul(h_sb[:, f * TOK_TILE: f * TOK_TILE + w],
                                         s_sb[:, :w], u_ps[:, :w])
                for dm in range(nd):
                    y_ps = yp.tile([128, TOK_TILE], F32)
                    for f in range(nf):
                        nc.tensor.matmul(
                            y_ps[:, :w],
                            wd_sb[f][:, ts(dm, 128)],
                            h_sb[:, f * TOK_TILE: f * TOK_TILE + w],
                            start=(f == 0), stop=(f == nf - 1))
                    y_sb = opool.tile([128, TOK_TILE], F32)
                    nc.vector.tensor_copy(y_sb[:, :w], y_ps[:, :w])
                    nc.sync.dma_start(yt[ts(dm, 128), off: off + w], y_sb[:, :w])
    nc.compile()
    return nc


def _route(x: np.ndarray, router_w: np.ndarray):
    """Host router identical in math to the jax reference (fp32)."""
    logits = x @ router_w.T                                   # [T, E]
    logits = logits - logits.max(axis=-1, keepdims=True)
    ex = np.exp(logits, dtype=np.float32)
    scores = ex / ex.sum(axis=-1, keepdims=True)              # [T, E]
    topk_idx = np.argsort(-scores, axis=-1, kind="stable")[:, :K]   # [T, K]
    topk_w = np.take_along_axis(scores, topk_idx, axis=-1)
    topk_w = topk_w / topk_w.sum(axis=-1, keepdims=True)
    return topk_idx.astype(np.int64), topk_w.astype(np.float32)


_NC_CACHE: dict[int, bass.Bass] = {}


def _run_device(in_maps, tcap, trace=False, **kw):
    nc = _NC_CACHE.get(tcap)
    if nc is None:
        nc = _build_nc(tcap)
        _NC_CACHE[tcap] = nc
    return run_bass_kernel_spmd(nc, in_maps, core_ids=list(range(N_CORES)),
                                trace=trace, **kw)


def _prepare(hidden_states, router_w, w_gate, w_up, w_down):
    x = np.ascontiguousarray(hidden_states.reshape(-1, D)).astype(np.float32)
    topk_idx, topk_w = _route(x, router_w.astype(np.float32))

    tok_lists, w_lists = [], []
    for e in range(E):
        mask = topk_idx == e                                   # [T, K]
        tok_e = np.nonzero(mask.any(axis=1))[0]
        w_e = (topk_w * mask)[tok_e].sum(axis=1).astype(np.float32)
        tok_lists.append(tok_e)
        w_lists.append(w_e)

    max_count = max(len(t) for t in tok_lists)
    tcap = -(-max_count // TOK_ALIGN) * TOK_ALIGN

    in_maps = []
    for e in range(E):
        xt = np.zeros((D, tcap), dtype=ACT_NP)
        xe = x[tok_lists[e]]                                   # [count, D] f32
        xt[:, :len(tok_lists[e])] = xe.T.astype(ACT_NP)
        in_maps.append({
            "xt": xt,
            "wg": np.ascontiguousarray(w_gate[e].T).astype(ACT_NP),
            "wu": np.ascontiguousarray(w_up[e].T).astype(ACT_NP),
            "wd": np.ascontiguousarray(w_down[e].T).astype(ACT_NP),
        })
    return in_maps, tok_lists, w_lists, tcap


def _combine(results, tok_lists, w_lists):
    out = np.zeros((B * S, D), dtype=np.float32)
    for e in range(E):
        yt = results[e]["yt"]                                  # [D, tcap] f32
        n = len(tok_lists[e])
        out[tok_lists[e]] += w_lists[e][:, None] * yt[:, :n].T
    return out.reshape(B, S, D)


def kernel(hidden_states, router_w, w_gate, w_up, w_down):
    in_maps, tok_lists, w_lists, tcap = _prepare(
        hidden_states, router_w, w_gate, w_up, w_down)
    res = _run_device(in_maps, tcap)
    return _combine(res.results, tok_lists, w_lists)


def kernel_traced(hidden_states, router_w, w_gate, w_up, w_down, **kw):
    """Same as kernel() but returns (output, BassKernelResults) with NTFF trace."""
    in_maps, tok_lists, w_lists, tcap = _prepare(
        hidden_states, router_w, w_gate, w_up, w_down)
    res = _run_device(in_maps, tcap, trace=True, **kw)
    return _combine(res.results, tok_lists, w_lists), res


# revision 13
# speedup vs baseline: 1.1372x; 1.0075x over previous
"""MoE (top-2 of 8 experts, SwiGLU MLP) Trainium2 kernel.

Strategy: expert parallelism across 8 NeuronCores. The (tiny) router runs
on host; tokens are gathered per expert on host and each core runs one
expert's SwiGLU MLP over its tokens with weights resident in SBUF.
Host applies the renormalized top-2 combine weights and scatter-adds the
two expert outputs per token.

Device layout: activations are kept transposed ([feature, token]) so every
matmul has its contraction dim on partitions with the weight tile
stationary; no on-device transposes are needed anywhere.
"""

import sys

import numpy as np

for _p in ("/root/.axon_site", "/root/.axon_site/_ro/trn_rl_repo",
           "/root/.axon_site/_ro/pypackages", "/opt/trn_rl_repo", "/opt/pypackages"):
    if _p not in sys.path:
        sys.path.append(_p)

import ml_dtypes  # noqa: E402

import concourse.bass as bass  # noqa: E402
import concourse.tile as tile  # noqa: E402
from concourse import bacc, mybir  # noqa: E402
from concourse.bass import ts  # noqa: E402
from concourse.bass_utils import run_bass_kernel_spmd  # noqa: E402

B, S, D, F, E, K = 4, 4096, 1024, 2048, 8, 2
N_CORES = 8
TOK_TILE = 512
TOK_ALIGN = 32
ACT_DT = mybir.dt.float16
ACT_NP = np.float16
F32 = mybir.dt.float32


def _build_nc(tcap: int) -> bass.Bass:
    """One expert's SwiGLU MLP over tcap (padded) tokens, transposed layout."""
    nd = D // 128   # 8 d-chunks
    nf = F // 128   # 16 f-chunks
    # full 512-token tiles; a short remainder is equalized across the last
    # two tiles (N=64 matmuls are NX-issue-bound at ~60ns, N>=256 are not)
    nfull, rem = divmod(tcap, TOK_TILE)
    widths = [TOK_TILE] * nfull
    if rem:
        if nfull:
            last_two = TOK_TILE + rem
            a = (last_two // 2 + 31) // 32 * 32
            widths = [TOK_TILE] * (nfull - 1) + [a, last_two - a]
        else:
            widths = [rem]
    tiles = []
    o = 0
    for w in widths:
        tiles.append((o, w))
        o += w

    nc = bacc.Bacc("TRN2", debug=False, target_bir_lowering=False,
                   num_devices=N_CORES)
    xt = nc.dram_tensor("xt", [D, tcap], ACT_DT, kind="ExternalInput").ap()
    wg = nc.dram_tensor("wg", [D, F], ACT_DT, kind="ExternalInput").ap()
    wu = nc.dram_tensor("wu", [D, F], ACT_DT, kind="ExternalInput").ap()
    wd = nc.dram_tensor("wd", [F, D], ACT_DT, kind="ExternalInput").ap()
    yt = nc.dram_tensor("yt", [D, tcap], F32, kind="ExternalOutput").ap()

    with tile.TileContext(nc) as tc:
        with tc.tile_pool(name="wpool", bufs=1) as wpool, \
             tc.tile_pool(name="xpool", bufs=3) as xpool, \
             tc.tile_pool(name="hpool", bufs=2) as hpool, \
             tc.tile_pool(name="spool", bufs=3) as spool, \
             tc.tile_pool(name="opool", bufs=3) as opool, \
             tc.tile_pool(name="gp", bufs=2, space="PSUM") as gp, \
             tc.tile_pool(name="up", bufs=2, space="PSUM") as up, \
             tc.tile_pool(name="yp", bufs=2, space="PSUM") as yp:

            # gate/up weights as per-chunk HALF tiles: the first accumulation
            # chains only need the f<8 halves, which land in ~half the DMA time
            FH = F // 2
            wg_sb = [[wpool.tile([128, FH], ACT_DT, name=f"wg{c}h{h}")
                      for h in range(2)] for c in range(nd)]
            wu_sb = [[wpool.tile([128, FH], ACT_DT, name=f"wu{c}h{h}")
                      for h in range(2)] for c in range(nd)]
            wd_sb = [wpool.tile([128, D], ACT_DT, name=f"wd_sb{c}") for c in range(nf)]
            # first x tile first, then first halves of wg/wu, then the rest
            x_tiles = {}
            x_tiles[0] = xpool.tile([128, nd * TOK_TILE], ACT_DT, tag="x", name="x_sb0")
            for c in range(nd):
                nc.sync.dma_start(x_tiles[0][:, c * TOK_TILE: c * TOK_TILE + tiles[0][1]],
                                  xt[ts(c, 128), tiles[0][0]: tiles[0][0] + tiles[0][1]])
            for c in range(nd):
                nc.sync.dma_start(wg_sb[c][0][:], wg[ts(c, 128), :FH])
            for c in range(nd):
                nc.sync.dma_start(wu_sb[c][0][:], wu[ts(c, 128), :FH])
            for c in range(nd):
                nc.sync.dma_start(wg_sb[c][1][:], wg[ts(c, 128), FH:])
            for c in range(nd):
                nc.sync.dma_start(wu_sb[c][1][:], wu[ts(c, 128), FH:])
            for c in range(nf):
                nc.sync.dma_start(wd_sb[c][:], wd[ts(c, 128), :])

            for j, (off, w) in enumerate(tiles):
                x_sb = x_tiles.get(j)
                if x_sb is None:
                    x_sb = xpool.tile([128, nd * TOK_TILE], ACT_DT, tag="x", name=f"x_sb{j}")
                    for c in range(nd):
                        nc.sync.dma_start(x_sb[:, c * TOK_TILE: c * TOK_TILE + w],
                                          xt[ts(c, 128), off: off + w])
                h_sb = hpool.tile([128, nf * TOK_TILE], ACT_DT)
                for f in range(nf):
                    g_ps = gp.tile([128, TOK_TILE], F32)
                    for c in range(nd):
                        nc.tensor.matmul(
                            g_ps[:, :w],
                            wg_sb[c][f // 8][:, ts(f % 8, 128)],
                            x_sb[:, c * TOK_TILE: c * TOK_TILE + w],
                            start=(c == 0), stop=(c == nd - 1))
                    u_ps = up.tile([128, TOK_TILE], F32)
                    for c in range(nd):
                        nc.tensor.matmul(
                            u_ps[:, :w],
                            wu_sb[c][f // 8][:, ts(f % 8, 128)],
                            x_sb[:, c * TOK_TILE: c * TOK_TILE + w],
                            start=(c == 0), stop=(c == nd - 1))
                    s_sb = spool.tile([128, TOK_TILE], F32)
                    nc.scalar.activation(s_sb[:, :w], g_ps[:, :w],
                                         mybir.ActivationFunctionType.Silu)
                    nc.vector.tensor_mul(h_sb[:, f * TOK_TILE: f * TOK_TILE + w],
                                         s_sb[:, :w], u_ps[:, :w])
                for dm in range(nd):
                    y_ps = yp.tile([128, TOK_TILE], F32)
                    for f in range(nf):
                        nc.tensor.matmul(
                            y_ps[:, :w],
                            wd_sb[f][:, ts(dm, 128)],
                            h_sb[:, f * TOK_TILE: f * TOK_TILE + w],
                            start=(f == 0), stop=(f == nf - 1))
                    y_sb = opool.tile([128, TOK_TILE], F32)
                    nc.vector.tensor_copy(y_sb[:, :w], y_ps[:, :w])
                    nc.sync.dma_start(yt[ts(dm, 128), off: off + w], y_sb[:, :w])
    nc.compile()
    return nc


def _route(x: np.ndarray, router_w: np.ndarray):
    """Host router identical in math to the jax reference (fp32)."""
    logits = x @ router_w.T                                   # [T, E]
    logits = logits - logits.max(axis=-1, keepdims=True)
    ex = np.exp(logits, dtype=np.float32)
    scores = ex / ex.sum(axis=-1, keepdims=True)              # [T, E]
    topk_idx = np.argsort(-scores, axis=-1, kind="stable")[:, :K]   # [T, K]
    topk_w = np.take_along_axis(scores, topk_idx, axis=-1)
    topk_w = topk_w / topk_w.sum(axis=-1, keepdims=True)
    return topk_idx.astype(np.int64), topk_w.astype(np.float32)


_NC_CACHE: dict[int, bass.Bass] = {}


def _run_device(in_maps, tcap, trace=False, **kw):
    nc = _NC_CACHE.get(tcap)
    if nc is None:
        nc = _build_nc(tcap)
        _NC_CACHE[tcap] = nc
    return run_bass_kernel_spmd(nc, in_maps, core_ids=list(range(N_CORES)),
                                trace=trace, **kw)


def _prepare(hidden_states, router_w, w_gate, w_up, w_down):
    x = np.ascontiguousarray(hidden_states.reshape(-1, D)).astype(np.float32)
    topk_idx, topk_w = _route(x, router_w.astype(np.float32))

    tok_lists, w_lists = [], []
    for e in range(E):
        mask = topk_idx == e                                   # [T, K]
        tok_e = np.nonzero(mask.any(axis=1))[0]
        w_e = (topk_w * mask)[tok_e].sum(axis=1).astype(np.float32)
        tok_lists.append(tok_e)
        w_lists.append(w_e)

    max_count = max(len(t) for t in tok_lists)
    tcap = -(-max_count // TOK_ALIGN) * TOK_ALIGN

    in_maps = []
    for e in range(E):
        xt = np.zeros((D, tcap), dtype=ACT_NP)
        xe = x[tok_lists[e]]                                   # [count, D] f32
        xt[:, :len(tok_lists[e])] = xe.T.astype(ACT_NP)
        in_maps.append({
            "xt": xt,
            "wg": np.ascontiguousarray(w_gate[e].T).astype(ACT_NP),
            "wu": np.ascontiguousarray(w_up[e].T).astype(ACT_NP),
            "wd": np.ascontiguousarray(w_down[e].T).astype(ACT_NP),
        })
    return in_maps, tok_lists, w_lists, tcap


def _combine(results, tok_lists, w_lists):
    out = np.zeros((B * S, D), dtype=np.float32)
    for e in range(E):
        yt = results[e]["yt"]                                  # [D, tcap] f32
        n = len(tok_lists[e])
        out[tok_lists[e]] += w_lists[e][:, None] * yt[:, :n].T
    return out.reshape(B, S, D)


def kernel(hidden_states, router_w, w_gate, w_up, w_down):
    in_maps, tok_lists, w_lists, tcap = _prepare(
        hidden_states, router_w, w_gate, w_up, w_down)
    res = _run_device(in_maps, tcap)
    return _combine(res.results, tok_lists, w_lists)


def kernel_traced(hidden_states, router_w, w_gate, w_up, w_down, **kw):
    """Same as kernel() but returns (output, BassKernelResults) with NTFF trace."""
    in_maps, tok_lists, w_lists, tcap = _prepare(
        hidden_states, router_w, w_gate, w_up, w_down)
    res = _run_device(in_maps, tcap, trace=True, **kw)
    return _combine(res.results, tok_lists, w_lists), res
